# revision 18
# baseline (speedup 1.0000x reference)
"""Mixture-of-Experts (T=1024, H=1024, F=2048, E=8, top-k=2) on 8 trn2 cores.

Strategy: expert parallelism. Core e owns expert e's weights. The host
gathers each expert's routed tokens (max ~283 for 2048 slots over 8
experts), pads to a fixed capacity C, and ships them transposed so the
whole device-side pipeline runs in a "feature-on-partition" layout:

    fc1:  h1T[4096, C] = w1[e] @ xT          (lhsT = w1[e].T chunks)
    swiglu: actT[2048, C] = silu(gateT + b1g) * (linT + b1l)
    fc2:  yT[1024, C] = w2[e] @ actT + b2

No on-chip transposes are needed, biases land on the partition dim, and
the host applies the per-slot final scales during the scatter-add
combine.

Matmuls run as float32r (fp32 data, PE truncates inputs to ~fp22/e10m11
and accumulates fp32) which streams at bf16 rate for moving dims >= 256.

The kernel is DMA-roofline-bound: 24MB of expert weights per core at
the measured ~280-300 GB/s effective HBM->SBUF bandwidth dominates; the
PE pipeline (384 LDWEIGHTS+MATMUL pairs at ~212ns) hides under the
weight stream. Weights are host-packed so every weight DMA is one large
contiguous transfer on the SP HWDGE ring; outputs and biases ride the
ACT HWDGE ring so they never delay the weight stream; the first
transfers are split finer so the PE starts ~5us in. Measured on HW
(NTFF profile, core 0): ~87-98us depending on chip clock/contention
state.
"""

import numpy as np
from contextlib import ExitStack

import concourse.bass as bass
import concourse.mybir as mybir
import concourse.tile as tile
from concourse import bacc
from concourse.bass_utils import run_bass_kernel_spmd

T, H, F, E, TOPK = 1024, 1024, 2048, 8, 2
P = 128
C_ALIGN = 4        # capacity rounding; actual C picked at runtime from routing
C_MAX = 512        # PSUM free-dim limit (2KB bank / 4B)
KH = H // P        # 8   fc1 contraction chunks
MG = F // P        # 16  gate m-chunks (lin chunks are MG..2MG-1)
KF = F // P        # 16  fc2 contraction chunks
M2 = H // P        # 8   fc2 output chunks
F32 = mybir.dt.float32
F32R = mybir.dt.float32r
BF16 = mybir.dt.bfloat16

import ml_dtypes
NP_BF16 = ml_dtypes.bfloat16

TRACE = False
TRACE_KWARGS = {}
LAST_RESULT = None

_nc_cache = {}


def _build_nc(C: int, repeat: int = 1) -> bass.Bass:
    nc = bacc.Bacc("TRN2", target_bir_lowering=False, debug=False)
    xs = nc.dram_tensor("xs", [P, KH, C], BF16, kind="ExternalInput")
    w1s = nc.dram_tensor("w1s", [MG // 2, 2, 2, P, KH, P], BF16, kind="ExternalInput")
    w2s = nc.dram_tensor("w2s", [M2 // 2, 2, P, KF, P], BF16, kind="ExternalInput")
    # b1 (32 per-partition columns) and b2 (8) merged into one small DMA
    bs = nc.dram_tensor("bs", [P, 2 * MG + M2], F32, kind="ExternalInput")
    ys = nc.dram_tensor("ys", [M2, P, C], BF16, kind="ExternalOutput")

    silu = mybir.ActivationFunctionType.Silu

    with tile.TileContext(nc) as tc, ExitStack() as ctx:
        consts = ctx.enter_context(tc.tile_pool(name="consts", bufs=1))
        xpool = ctx.enter_context(tc.tile_pool(name="xpool", bufs=1))
        w1pool = ctx.enter_context(tc.tile_pool(name="w1pool", bufs=4))
        w2pool = ctx.enter_context(tc.tile_pool(name="w2pool", bufs=3))
        actpool = ctx.enter_context(tc.tile_pool(name="actpool", bufs=1))
        evpool = ctx.enter_context(tc.tile_pool(name="evpool", bufs=4))
        ypool = ctx.enter_context(tc.tile_pool(name="ypool", bufs=3))
        ps1 = ctx.enter_context(tc.tile_pool(name="ps1", bufs=4, space="PSUM"))
        ps2 = ctx.enter_context(tc.tile_pool(name="ps2", bufs=2, space="PSUM"))
        pswarm = ctx.enter_context(tc.tile_pool(name="pswarm", bufs=1, space="PSUM"))

        for _rep in range(repeat):
            # PE p-state warmup: dependency-free matmuls on a memset-zero
            # tile fill the DMA-wait window so the real matmuls start at
            # full clock instead of paying the ~3us ramp.
            warm = consts.tile([P, 256], BF16)
            nc.gpsimd.memset(warm, 0)
            pwarm = pswarm.tile([P, 256], F32)
            for i in range(18):
                nc.tensor.matmul(
                    pwarm, lhsT=warm[:, :P], rhs=warm, start=(i == 0), stop=(i == 17)
                )

            # Startup: first-group deps split across rings — x on SP, the
            # two s=0 w1 slices + bias on ACT — so the SP ring reaches the
            # bulk w1 stream (s=1 slice, then jj>=1 chunks) as early as
            # possible.
            x_a = xpool.tile([P, KH // 2, C], BF16, tag="xa")
            nc.sync.dma_start(out=x_a, in_=xs[:, : KH // 2, :])
            w1_first = w1pool.tile([P, 2, 2, KH, P], BF16, tag="w1")
            nc.scalar.dma_start(out=w1_first[:, 0, 0], in_=w1s[0, 0, 0])
            x_b = xpool.tile([P, KH // 2, C], BF16, tag="xb")
            nc.sync.dma_start(out=x_b, in_=xs[:, KH // 2 :, :])
            nc.scalar.dma_start(out=w1_first[:, 0, 1], in_=w1s[0, 0, 1])
            nc.sync.dma_start(
                out=w1_first[:, 1], in_=w1s[0, 1].rearrange("g p k n -> p g k n")
            )

            def xk(k):
                return x_a[:, k, :] if k < KH // 2 else x_b[:, k - KH // 2, :]
            b_sb = consts.tile([P, 2 * MG + M2], F32)
            nc.scalar.dma_start(out=b_sb, in_=bs[:, :])
            b1_sb = b_sb[:, : 2 * MG]
            b2_sb = b_sb[:, 2 * MG :]

            act_all = actpool.tile([P, KF, C], BF16)

            # fc1 + swiglu: each outer iteration streams one 1MB weight
            # chunk holding gate/lin m-chunk pairs (2*jj+s, 16+2*jj+s).
            for jj in range(MG // 2):
                if jj == 0:
                    w1_sb = w1_first
                else:
                    w1_sb = w1pool.tile([P, 2, 2, KH, P], BF16, tag="w1")
                    nc.sync.dma_start(
                        out=w1_sb, in_=w1s[jj].rearrange("s g p k n -> p s g k n")
                    )
                for s in range(2):
                    m = 2 * jj + s
                    pg = ps1.tile([P, C], F32, tag="ps1")
                    pl = ps1.tile([P, C], F32, tag="ps1")
                    for k in range(KH):
                        nc.tensor.matmul(
                            pg,
                            lhsT=w1_sb[:, s, 0, k, :],
                            rhs=xk(k),
                            start=(k == 0),
                            stop=(k == KH - 1),
                        )
                    for k in range(KH):
                        nc.tensor.matmul(
                            pl,
                            lhsT=w1_sb[:, s, 1, k, :],
                            rhs=xk(k),
                            start=(k == 0),
                            stop=(k == KH - 1),
                        )
                    gate_sb = evpool.tile([P, C], F32, tag="gate")
                    nc.scalar.activation(gate_sb, pg, silu, bias=b1_sb[:, m : m + 1])
                    lin_sb = evpool.tile([P, C], F32, tag="lin")
                    nc.vector.tensor_scalar_add(lin_sb, pl, b1_sb[:, MG + m : MG + m + 1])
                    nc.vector.tensor_mul(act_all[:, m, :], gate_sb, lin_sb)

            # fc2: stream 1MB chunks holding output m-chunk pairs.
            for mm in range(M2 // 2):
                w2_sb = w2pool.tile([P, 2, KF, P], BF16, tag="w2")
                if mm == M2 // 2 - 1:
                    nc.sync.dma_start(
                        out=w2_sb[:, 0], in_=w2s[mm, 0].rearrange("p k n -> p k n")
                    )
                    nc.sync.dma_start(
                        out=w2_sb[:, 1], in_=w2s[mm, 1].rearrange("p k n -> p k n")
                    )
                else:
                    nc.sync.dma_start(
                        out=w2_sb, in_=w2s[mm].rearrange("s p k n -> p s k n")
                    )
                y_sb = ypool.tile([P, 2, C], BF16, tag="y")
                for s in range(2):
                    m = 2 * mm + s
                    p2 = ps2.tile([P, C], F32, tag="ps2")
                    for k in range(KF):
                        nc.tensor.matmul(
                            p2,
                            lhsT=w2_sb[:, s, k, :],
                            rhs=act_all[:, k, :],
                            start=(k == 0),
                            stop=(k == KF - 1),
                        )
                    if mm == M2 // 2 - 1 and s == 1:
                        # final slice: evacuate in two halves across both
                        # rings so the first DMA issue overlaps the second
                        # bias-add, shortening the tail
                        hc = C // 2
                        nc.vector.tensor_scalar_add(
                            y_sb[:, s, :hc], p2[:, :hc], b2_sb[:, m : m + 1]
                        )
                        nc.scalar.dma_start(out=ys[m][:, :hc], in_=y_sb[:, s, :hc])
                        nc.vector.tensor_scalar_add(
                            y_sb[:, s, hc:], p2[:, hc:], b2_sb[:, m : m + 1]
                        )
                        nc.sync.dma_start(out=ys[m][:, hc:], in_=y_sb[:, s, hc:])
                    else:
                        nc.vector.tensor_scalar_add(
                            y_sb[:, s, :], p2, b2_sb[:, m : m + 1]
                        )
                        # outputs ride the ACT ring so they never delay the
                        # weight stream on the SP ring
                        nc.scalar.dma_start(out=ys[m], in_=y_sb[:, s, :])

    nc.compile()
    return nc


def _get_nc(C: int) -> bass.Bass:
    if C not in _nc_cache:
        _nc_cache[C] = _build_nc(C)
    return _nc_cache[C]


def _pack_weights(w1, b1, w2, b2):
    """Per-expert host packing into the DMA-friendly layouts."""
    packed = []
    for e in range(E):
        # [m, p, k, n] with lhsT[p, n] = w[m*128+n, k*128+p]
        w1c = np.ascontiguousarray(
            w1[e].astype(NP_BF16).reshape(2 * MG, P, KH, P).transpose(0, 3, 2, 1)
        )
        w1se = np.ascontiguousarray(
            np.stack(
                [
                    w1c[:MG].reshape(MG // 2, 2, P, KH, P),
                    w1c[MG:].reshape(MG // 2, 2, P, KH, P),
                ],
                axis=2,
            )
        )
        w2c = w2[e].astype(NP_BF16).reshape(M2, P, KF, P).transpose(0, 3, 2, 1)
        w2se = np.ascontiguousarray(w2c.reshape(M2 // 2, 2, P, KF, P))
        bse = np.ascontiguousarray(
            np.concatenate([b1[e].reshape(2 * MG, P), b2[e].reshape(M2, P)], 0).T
        )
        packed.append((w1se, w2se, bse))
    return packed


def kernel(
    hidden_states,
    token_selected_experts,
    token_final_scales,
    w1,
    b1,
    w2,
    b2,
):
    global LAST_RESULT
    hs = np.ascontiguousarray(np.asarray(hidden_states, dtype=np.float32))
    sel = np.asarray(token_selected_experts, dtype=np.int32)
    scl = np.asarray(token_final_scales, dtype=np.float32)
    w1 = np.asarray(w1, dtype=np.float32)
    b1 = np.asarray(b1, dtype=np.float32)
    w2 = np.asarray(w2, dtype=np.float32)
    b2 = np.asarray(b2, dtype=np.float32)

    nt, hh = hs.shape
    assert (nt, hh) == (T, H), f"unexpected shape {hs.shape}"

    # Route: stable-sort the (token, k) slots by selected expert.
    flat_e = sel.reshape(-1)
    slot_tok = np.repeat(np.arange(T, dtype=np.int64), TOPK)
    order = np.argsort(flat_e, kind="stable")
    sorted_tok = slot_tok[order]
    sorted_scl = scl.reshape(-1)[order]
    counts = np.bincount(flat_e, minlength=E)
    starts = np.concatenate([[0], np.cumsum(counts)])
    # capacity sized to the observed max bucket (multi-launch fallback
    # keeps any larger routing correct, just slower)
    C = min(C_MAX, -(-max(1, int(counts.max())) // C_ALIGN) * C_ALIGN)
    n_chunks = max(1, -(-int(counts.max()) // C))

    packed = _pack_weights(w1, b1, w2, b2)
    nc = _get_nc(C)

    out = np.zeros((T, H), dtype=np.float32)
    for ci in range(n_chunks):
        in_maps = []
        metas = []
        for e in range(E):
            lo = int(starts[e]) + ci * C
            hi = min(int(starts[e + 1]), lo + C)
            ids = sorted_tok[lo:hi] if hi > lo else np.empty(0, np.int64)
            n = len(ids)
            xg = np.zeros((C, H), dtype=NP_BF16)
            if n:
                xg[:n] = hs[ids].astype(NP_BF16)
            xse = np.ascontiguousarray(xg.T.reshape(KH, P, C).transpose(1, 0, 2))
            w1se, w2se, bse = packed[e]
            in_maps.append({"xs": xse, "w1s": w1se, "w2s": w2se, "bs": bse})
            metas.append((ids, sorted_scl[lo:hi] if n else None))

        res = run_bass_kernel_spmd(
            nc,
            in_maps,
            core_ids=list(range(E)),
            trace=TRACE,
            **TRACE_KWARGS,
        )
        LAST_RESULT = res
        for e in range(E):
            ids, ss = metas[e]
            if ids is None or len(ids) == 0:
                continue
            yt = res.results[e]["ys"].reshape(H, C).astype(np.float32)
            contrib = yt[:, : len(ids)].T * ss[:, None]
            np.add.at(out, ids, contrib)

    return out



# revision 19
# speedup vs baseline: 1.0191x; 1.0191x over previous
"""Mixture-of-Experts (T=1024, H=1024, F=2048, E=8, top-k=2) on 8 trn2 cores.

Strategy: expert parallelism. Core e owns expert e's weights. The host
gathers each expert's routed tokens (max ~283 for 2048 slots over 8
experts), pads to a fixed capacity C, and ships them transposed so the
whole device-side pipeline runs in a "feature-on-partition" layout:

    fc1:  h1T[4096, C] = w1[e] @ xT          (lhsT = w1[e].T chunks)
    swiglu: actT[2048, C] = silu(gateT + b1g) * (linT + b1l)
    fc2:  yT[1024, C] = w2[e] @ actT + b2

No on-chip transposes are needed, biases land on the partition dim, and
the host applies the per-slot final scales during the scatter-add
combine.

Matmuls run as float32r (fp32 data, PE truncates inputs to ~fp22/e10m11
and accumulates fp32) which streams at bf16 rate for moving dims >= 256.

The kernel is DMA-roofline-bound: 24MB of expert weights per core at
the measured ~280-300 GB/s effective HBM->SBUF bandwidth dominates; the
PE pipeline (384 LDWEIGHTS+MATMUL pairs at ~212ns) hides under the
weight stream. Weights are host-packed so every weight DMA is one large
contiguous transfer on the SP HWDGE ring; outputs and biases ride the
ACT HWDGE ring so they never delay the weight stream; the first
transfers are split finer so the PE starts ~5us in. Measured on HW
(NTFF profile, core 0): ~87-98us depending on chip clock/contention
state.
"""

import numpy as np
from contextlib import ExitStack

import concourse.bass as bass
import concourse.mybir as mybir
import concourse.tile as tile
from concourse import bacc
from concourse.bass_utils import run_bass_kernel_spmd

T, H, F, E, TOPK = 1024, 1024, 2048, 8, 2
P = 128
C_ALIGN = 4        # capacity rounding; actual C picked at runtime from routing
C_MAX = 512        # PSUM free-dim limit (2KB bank / 4B)
KH = H // P        # 8   fc1 contraction chunks
MG = F // P        # 16  gate m-chunks (lin chunks are MG..2MG-1)
KF = F // P        # 16  fc2 contraction chunks
M2 = H // P        # 8   fc2 output chunks
F32 = mybir.dt.float32
F32R = mybir.dt.float32r
BF16 = mybir.dt.bfloat16

import ml_dtypes
NP_BF16 = ml_dtypes.bfloat16

TRACE = False
TRACE_KWARGS = {}
LAST_RESULT = None

_nc_cache = {}


def _build_nc(C: int, repeat: int = 1) -> bass.Bass:
    nc = bacc.Bacc("TRN2", target_bir_lowering=False, debug=False)
    xs = nc.dram_tensor("xs", [P, KH, C], BF16, kind="ExternalInput")
    w1s = nc.dram_tensor("w1s", [MG // 2, 2, 2, P, KH, P], BF16, kind="ExternalInput")
    w2s = nc.dram_tensor("w2s", [M2 // 2, 2, P, KF, P], BF16, kind="ExternalInput")
    # b1 (32 per-partition columns) and b2 (8) merged into one small DMA
    bs = nc.dram_tensor("bs", [P, 2 * MG + M2], F32, kind="ExternalInput")
    ys = nc.dram_tensor("ys", [M2, P, C], BF16, kind="ExternalOutput")

    silu = mybir.ActivationFunctionType.Silu

    with tile.TileContext(nc) as tc, ExitStack() as ctx:
        consts = ctx.enter_context(tc.tile_pool(name="consts", bufs=1))
        xpool = ctx.enter_context(tc.tile_pool(name="xpool", bufs=1))
        w1pool = ctx.enter_context(tc.tile_pool(name="w1pool", bufs=4))
        w2pool = ctx.enter_context(tc.tile_pool(name="w2pool", bufs=3))
        actpool = ctx.enter_context(tc.tile_pool(name="actpool", bufs=1))
        evpool = ctx.enter_context(tc.tile_pool(name="evpool", bufs=4))
        ypool = ctx.enter_context(tc.tile_pool(name="ypool", bufs=3))
        ps1 = ctx.enter_context(tc.tile_pool(name="ps1", bufs=4, space="PSUM"))
        ps2 = ctx.enter_context(tc.tile_pool(name="ps2", bufs=2, space="PSUM"))
        pswarm = ctx.enter_context(tc.tile_pool(name="pswarm", bufs=1, space="PSUM"))

        for _rep in range(repeat):
            # PE p-state warmup: dependency-free matmuls on a memset-zero
            # tile fill the DMA-wait window so the real matmuls start at
            # full clock instead of paying the ~3us ramp.
            warm = consts.tile([P, 256], BF16)
            nc.gpsimd.memset(warm, 0)
            pwarm = pswarm.tile([P, 256], F32)
            for i in range(18):
                nc.tensor.matmul(
                    pwarm, lhsT=warm[:, :P], rhs=warm, start=(i == 0), stop=(i == 17)
                )

            # Startup: q1 (SP) carries the first-group critical path (x
            # first half, gate m0 slice, then straight into the bulk w1
            # stream); q10 (ACT) takes x second half, the lin m0 slice and
            # bias so q1 reaches the jj=1 chunk before the PE needs it.
            x_a = xpool.tile([P, KH // 2, C], BF16, tag="xa")
            nc.sync.dma_start(out=x_a, in_=xs[:, : KH // 2, :])
            x_b = xpool.tile([P, KH // 2, C], BF16, tag="xb")
            nc.scalar.dma_start(out=x_b, in_=xs[:, KH // 2 :, :])
            w1_first = w1pool.tile([P, 2, 2, KH, P], BF16, tag="w1")
            nc.sync.dma_start(out=w1_first[:, 0, 0], in_=w1s[0, 0, 0])
            nc.scalar.dma_start(out=w1_first[:, 0, 1], in_=w1s[0, 0, 1])
            nc.sync.dma_start(
                out=w1_first[:, 1], in_=w1s[0, 1].rearrange("g p k n -> p g k n")
            )

            def xk(k):
                return x_a[:, k, :] if k < KH // 2 else x_b[:, k - KH // 2, :]
            b_sb = consts.tile([P, 2 * MG + M2], F32)
            nc.scalar.dma_start(out=b_sb, in_=bs[:, :])
            b1_sb = b_sb[:, : 2 * MG]
            b2_sb = b_sb[:, 2 * MG :]

            act_all = actpool.tile([P, KF, C], BF16)

            # fc1 + swiglu: each outer iteration streams one 1MB weight
            # chunk holding gate/lin m-chunk pairs (2*jj+s, 16+2*jj+s).
            for jj in range(MG // 2):
                if jj == 0:
                    w1_sb = w1_first
                else:
                    w1_sb = w1pool.tile([P, 2, 2, KH, P], BF16, tag="w1")
                    nc.sync.dma_start(
                        out=w1_sb, in_=w1s[jj].rearrange("s g p k n -> p s g k n")
                    )
                for s in range(2):
                    m = 2 * jj + s
                    pg = ps1.tile([P, C], F32, tag="ps1")
                    pl = ps1.tile([P, C], F32, tag="ps1")
                    for k in range(KH):
                        nc.tensor.matmul(
                            pg,
                            lhsT=w1_sb[:, s, 0, k, :],
                            rhs=xk(k),
                            start=(k == 0),
                            stop=(k == KH - 1),
                        )
                    for k in range(KH):
                        nc.tensor.matmul(
                            pl,
                            lhsT=w1_sb[:, s, 1, k, :],
                            rhs=xk(k),
                            start=(k == 0),
                            stop=(k == KH - 1),
                        )
                    gate_sb = evpool.tile([P, C], F32, tag="gate")
                    nc.scalar.activation(gate_sb, pg, silu, bias=b1_sb[:, m : m + 1])
                    lin_sb = evpool.tile([P, C], F32, tag="lin")
                    nc.vector.tensor_scalar_add(lin_sb, pl, b1_sb[:, MG + m : MG + m + 1])
                    nc.vector.tensor_mul(act_all[:, m, :], gate_sb, lin_sb)

            # fc2: stream 1MB chunks holding output m-chunk pairs.
            for mm in range(M2 // 2):
                w2_sb = w2pool.tile([P, 2, KF, P], BF16, tag="w2")
                if mm == M2 // 2 - 1:
                    nc.sync.dma_start(
                        out=w2_sb[:, 0], in_=w2s[mm, 0].rearrange("p k n -> p k n")
                    )
                    nc.sync.dma_start(
                        out=w2_sb[:, 1], in_=w2s[mm, 1].rearrange("p k n -> p k n")
                    )
                else:
                    nc.sync.dma_start(
                        out=w2_sb, in_=w2s[mm].rearrange("s p k n -> p s k n")
                    )
                y_sb = ypool.tile([P, 2, C], BF16, tag="y")
                for s in range(2):
                    m = 2 * mm + s
                    p2 = ps2.tile([P, C], F32, tag="ps2")
                    for k in range(KF):
                        nc.tensor.matmul(
                            p2,
                            lhsT=w2_sb[:, s, k, :],
                            rhs=act_all[:, k, :],
                            start=(k == 0),
                            stop=(k == KF - 1),
                        )
                    if mm == M2 // 2 - 1 and s == 1:
                        # final slice: evacuate in two halves across both
                        # rings so the first DMA issue overlaps the second
                        # bias-add, shortening the tail
                        hc = C // 2
                        nc.vector.tensor_scalar_add(
                            y_sb[:, s, :hc], p2[:, :hc], b2_sb[:, m : m + 1]
                        )
                        nc.scalar.dma_start(out=ys[m][:, :hc], in_=y_sb[:, s, :hc])
                        nc.vector.tensor_scalar_add(
                            y_sb[:, s, hc:], p2[:, hc:], b2_sb[:, m : m + 1]
                        )
                        nc.sync.dma_start(out=ys[m][:, hc:], in_=y_sb[:, s, hc:])
                    else:
                        nc.vector.tensor_scalar_add(
                            y_sb[:, s, :], p2, b2_sb[:, m : m + 1]
                        )
                        # outputs ride the ACT ring so they never delay the
                        # weight stream on the SP ring
                        nc.scalar.dma_start(out=ys[m], in_=y_sb[:, s, :])

    nc.compile()
    return nc


def _get_nc(C: int) -> bass.Bass:
    if C not in _nc_cache:
        _nc_cache[C] = _build_nc(C)
    return _nc_cache[C]


def _pack_weights(w1, b1, w2, b2):
    """Per-expert host packing into the DMA-friendly layouts."""
    packed = []
    for e in range(E):
        # [m, p, k, n] with lhsT[p, n] = w[m*128+n, k*128+p]
        w1c = np.ascontiguousarray(
            w1[e].astype(NP_BF16).reshape(2 * MG, P, KH, P).transpose(0, 3, 2, 1)
        )
        w1se = np.ascontiguousarray(
            np.stack(
                [
                    w1c[:MG].reshape(MG // 2, 2, P, KH, P),
                    w1c[MG:].reshape(MG // 2, 2, P, KH, P),
                ],
                axis=2,
            )
        )
        w2c = w2[e].astype(NP_BF16).reshape(M2, P, KF, P).transpose(0, 3, 2, 1)
        w2se = np.ascontiguousarray(w2c.reshape(M2 // 2, 2, P, KF, P))
        bse = np.ascontiguousarray(
            np.concatenate([b1[e].reshape(2 * MG, P), b2[e].reshape(M2, P)], 0).T
        )
        packed.append((w1se, w2se, bse))
    return packed


def kernel(
    hidden_states,
    token_selected_experts,
    token_final_scales,
    w1,
    b1,
    w2,
    b2,
):
    global LAST_RESULT
    hs = np.ascontiguousarray(np.asarray(hidden_states, dtype=np.float32))
    sel = np.asarray(token_selected_experts, dtype=np.int32)
    scl = np.asarray(token_final_scales, dtype=np.float32)
    w1 = np.asarray(w1, dtype=np.float32)
    b1 = np.asarray(b1, dtype=np.float32)
    w2 = np.asarray(w2, dtype=np.float32)
    b2 = np.asarray(b2, dtype=np.float32)

    nt, hh = hs.shape
    assert (nt, hh) == (T, H), f"unexpected shape {hs.shape}"

    # Route: stable-sort the (token, k) slots by selected expert.
    flat_e = sel.reshape(-1)
    slot_tok = np.repeat(np.arange(T, dtype=np.int64), TOPK)
    order = np.argsort(flat_e, kind="stable")
    sorted_tok = slot_tok[order]
    sorted_scl = scl.reshape(-1)[order]
    counts = np.bincount(flat_e, minlength=E)
    starts = np.concatenate([[0], np.cumsum(counts)])
    # capacity sized to the observed max bucket (multi-launch fallback
    # keeps any larger routing correct, just slower)
    C = min(C_MAX, -(-max(1, int(counts.max())) // C_ALIGN) * C_ALIGN)
    n_chunks = max(1, -(-int(counts.max()) // C))

    packed = _pack_weights(w1, b1, w2, b2)
    nc = _get_nc(C)

    out = np.zeros((T, H), dtype=np.float32)
    for ci in range(n_chunks):
        in_maps = []
        metas = []
        for e in range(E):
            lo = int(starts[e]) + ci * C
            hi = min(int(starts[e + 1]), lo + C)
            ids = sorted_tok[lo:hi] if hi > lo else np.empty(0, np.int64)
            n = len(ids)
            xg = np.zeros((C, H), dtype=NP_BF16)
            if n:
                xg[:n] = hs[ids].astype(NP_BF16)
            xse = np.ascontiguousarray(xg.T.reshape(KH, P, C).transpose(1, 0, 2))
            w1se, w2se, bse = packed[e]
            in_maps.append({"xs": xse, "w1s": w1se, "w2s": w2se, "bs": bse})
            metas.append((ids, sorted_scl[lo:hi] if n else None))

        res = run_bass_kernel_spmd(
            nc,
            in_maps,
            core_ids=list(range(E)),
            trace=TRACE,
            **TRACE_KWARGS,
        )
        LAST_RESULT = res
        for e in range(E):
            ids, ss = metas[e]
            if ids is None or len(ids) == 0:
                continue
            yt = res.results[e]["ys"].reshape(H, C).astype(np.float32)
            contrib = yt[:, : len(ids)].T * ss[:, None]
            np.add.at(out, ids, contrib)

    return out

